# revision 36
# baseline (speedup 1.0000x reference)
"""Trainium2 Bass kernel for nn_ContrastLoss (supervised-contrastive loss).

Reference computation (B=1024, D=128, C=100, K=32768, N=B+K=33792):
    l   = concat(labels, queue_label.T)          # [N, C]
    w   = labels @ l.T                           # [B, N] shared-class counts
    sim = query @ concat(keys, queue.T).T / T    # [B, N]
    logits = sim - rowmax(sim)
    denom  = sum(exp(logits) * logits_mask, 1)   # logits_mask zeros keys-diag
    loss = -(T/BT) * sqrt(w/max(w)) * (logits - log(denom))

Restructurings:
  * max(w) == max_i rowsum(labels_i) exactly (binary labels, diag of the
    keys block included): two tiny ones-vector matmuls + a free-dim max.
  * Constant softmax stabilizer m=1.0 (inputs are L2-normalized); the
    shift cancels in log_prob exactly.
  * qT host-scaled by -1/T so the sim matmul yields r = -raw/T and
    loss = s * (r + lnb), lnb = ln(denom') + m/T,
    s = sqrt(w*(T/BT)^2/wmax) >= 0 (folds sign and the w>0 mask).
    The elementwise tail is ONE all-bf16 scalar_tensor_tensor per chunk.
  * Output returned from device as bf16 (0.4% rel, gate is 2%): halves
    the dominant output DMA; host converts to f32.

Sharding: pure data-parallel over B -- core c owns rows [c*128,(c+1)*128)
and all N columns: NO collectives, immune to multi-core launch stagger.

Schedule notes (from NTFF traces): per-queue DMA sustains only ~115-160
GB/s, so inputs/outputs are spread over 4 queues (tensor/scalar queues
only carry dep-free transfers -- a dependent DMA's semaphore wait would
stall that engine's whole instruction stream).  2048-col chunks halve
the per-instruction semaphore overhead; one shared PSUM pool (2 x 4
banks) serves both phases.  A scheduler wait-hint keeps phase-B w
matmuls out of the sim-matmul window, because the sim chain feeds lnb
which gates every output STT.
"""

import numpy as np
import ml_dtypes

import concourse.bass as bass
import concourse.mybir as mybir
import concourse.tile as tile
from concourse import bacc
from concourse.bass_utils import run_bass_kernel_spmd

F32 = mybir.dt.float32
BF16 = mybir.dt.bfloat16
F8 = mybir.dt.float8e4
ALU = mybir.AluOpType
ACTF = mybir.ActivationFunctionType
AXX = mybir.AxisListType.X

B, D, C, KQ = 1024, 128, 100, 32768
NCORES = 8
RPC = B // NCORES          # 128 query rows per core
N = B + KQ                 # 33792 similarity columns, all on every core
CH = 2048                  # column chunk (psum tile: 4 banks)
SP = 4096                  # rhs_sim DMA piece (8KB bf16 partition lines)
RP = 4096                  # raw piece (cols)
WP = 8192                  # rhs_w DMA piece (8KB fp8 partition lines)
STAB = 1.0                 # constant softmax stabilizer


def _pieces(total, size):
    return [(a, min(a + size, total)) for a in range(0, total, size)]


def _build_nc(Tf: float, BTf: float):
    nc = bacc.Bacc("TRN2", target_bir_lowering=False, debug=False,
                   num_devices=NCORES)

    qT_d = nc.dram_tensor("qT", [D, RPC], BF16, kind="ExternalInput")
    rhs_sim_d = nc.dram_tensor("rhs_sim", [D, N], BF16, kind="ExternalInput")
    labT_d = nc.dram_tensor("labT", [C, RPC], F8, kind="ExternalInput")
    rhs_w_d = nc.dram_tensor("rhs_w", [C, N], F8, kind="ExternalInput")
    dmask_d = nc.dram_tensor("dmask", [RPC, B], BF16, kind="ExternalInput")
    out_d = nc.dram_tensor("out", [RPC, N], BF16, kind="ExternalOutput")

    # finer first pieces cut the wait for the second piece at kernel start
    sim_pieces = ([(a, a + 2048) for a in range(0, 8192, 2048)]
                  + _pieces(N, SP)[2:])
    raw_pieces = _pieces(N, RP)     # 9
    w_pieces = _pieces(N, WP)       # 5: 4x8192 + 1024
    chunks = _pieces(N, CH)         # 17: 16x2048 + 1024

    with tile.TileContext(nc) as tc:
        with (
            tc.tile_pool(name="const", bufs=1) as const,
            tc.tile_pool(name="simp", bufs=7) as simp,
            tc.tile_pool(name="spool", bufs=3) as spool,
            tc.tile_pool(name="stg", bufs=4) as stg,
            tc.tile_pool(name="ps2", bufs=2, space="PSUM") as ps2,
        ):
            # ---- input DMAs spread over 4 queues --------------------------
            # tensor/scalar queues get only dep-free transfers (their WAR
            # waits would stall the compute streams).
            # Only sync (SP) and scalar (Activation) are HW DGE queues; the
            # gpsimd queue is the slow software DGE (~60-100 GB/s), so it
            # gets only the late-needed rhs_w / dmask, while the latency-
            # critical rhs_sim pieces ride the two HW queues.
            qTc = const.tile([D, RPC], BF16)
            nc.sync.dma_start(out=qTc[:], in_=qT_d[:])
            labTc = const.tile([C, RPC], F8)
            nc.gpsimd.dma_start(out=labTc[:], in_=labT_d[:])

            wt = []
            for pi, (a, b) in enumerate(w_pieces):
                wt.append(const.tile([C, b - a], F8, name=f"wt{pi}"))
            nc.gpsimd.dma_start(out=wt[0][:], in_=rhs_w_d[:, 0:WP])

            st_tiles = []
            for pi, (a, b) in enumerate(sim_pieces):
                t = simp.tile([D, b - a], BF16, tag="sp", name=f"sp{pi}")
                eng = nc.scalar if pi % 2 == 0 else nc.sync
                eng.dma_start(out=t[:], in_=rhs_sim_d[:, a:b])
                st_tiles.append(t)

            dmask = const.tile([RPC, B], BF16)
            nc.gpsimd.dma_start(out=dmask[:], in_=dmask_d[:])
            for pi, (a, b) in enumerate(w_pieces):
                if pi == 0:
                    continue
                nc.gpsimd.dma_start(out=wt[pi][:], in_=rhs_w_d[:, a:b])

            ebias = const.tile([RPC, 1], F32)
            nc.vector.memset(ebias, -STAB / Tf)

            # ---- phase A: sim matmuls, DVE copy psum -> bf16 r = -raw/T ---
            raw = []
            for pi, (a, b) in enumerate(raw_pieces):
                raw.append(const.tile([D, b - a], BF16, name=f"raw{pi}"))
            for (a2, b2) in chunks:
                w2 = b2 - a2
                ps = ps2.tile([RPC, CH], F32, tag="ps")
                for o in range(0, w2, 512):
                    a = a2 + o
                    sp = next(i for i, (pa, pb) in enumerate(sim_pieces)
                              if pa <= a < pb)
                    off = a - sim_pieces[sp][0]
                    nc.tensor.matmul(ps[:, o:o + 512], qTc[:],
                                     st_tiles[sp][:, off:off + 512],
                                     start=True, stop=True)
                rp = a2 // RP
                ro = a2 - raw_pieces[rp][0]
                nc.vector.tensor_scalar_add(raw[rp][:, ro:ro + w2],
                                            ps[:, 0:w2], 0.0)

            # ---- all Exps (4096-col chunks over r), accum_out row sums ----
            e_keys = const.tile([RPC, B], F32)
            e_scr = const.tile([RPC, RP], BF16)
            acc = const.tile([RPC, len(raw_pieces) + 1], F32)
            nc.scalar.activation(e_keys[:], raw[0][:, 0:B], ACTF.Exp,
                                 bias=ebias[:], scale=-1.0,
                                 accum_out=acc[:, 0:1])
            nc.scalar.activation(e_scr[:, 0:RP - B], raw[0][:, B:RP], ACTF.Exp,
                                 bias=ebias[:], scale=-1.0,
                                 accum_out=acc[:, 1:2])
            for pi in range(1, len(raw_pieces)):
                a, b = raw_pieces[pi]
                nc.scalar.activation(e_scr[:, 0:b - a], raw[pi][:], ACTF.Exp,
                                     bias=ebias[:], scale=-1.0,
                                     accum_out=acc[:, pi + 1:pi + 2])

            # ---- denominator: subtract self-diagonal, take ln -------------
            nc.vector.tensor_mul(e_keys[:], e_keys[:], dmask[:])
            corr = const.tile([RPC, 1], F32)
            nc.vector.tensor_reduce(corr[:], e_keys[:], axis=AXX, op=ALU.add)
            dn = const.tile([RPC, 1], F32)
            nc.vector.tensor_reduce(dn[:], acc[:], axis=AXX, op=ALU.add)
            dn2 = const.tile([RPC, 1], F32)
            nc.vector.tensor_sub(dn2[:], dn[:], corr[:])
            lnd = const.tile([RPC, 1], F32)
            nc.scalar.activation(lnd[:], dn2[:], ACTF.Ln)
            lnb = const.tile([RPC, 1], F32)
            nc.vector.tensor_scalar_add(lnb[:], lnd[:], STAB / Tf)

            # ---- wmax = max_i rowsum(labels_i), via two tiny matmuls ------
            ones_c = const.tile([C, 1], F8)
            nc.vector.memset(ones_c, 1.0)
            ones_r = const.tile([1, RPC], F32)
            nc.vector.memset(ones_r, 1.0)
            pm = ps2.tile([RPC, CH], F32, tag="ps")
            nc.tensor.matmul(pm[0:1, 0:512], ones_c[:], wt[0][:, 0:512],
                             start=True, stop=True)
            nc.tensor.matmul(pm[0:1, 512:B], ones_c[:], wt[0][:, 512:B],
                             start=True, stop=True)
            wm = const.tile([1, 1], F32)
            nc.vector.tensor_reduce(wm[:], pm[0:1, 0:B], axis=AXX, op=ALU.max)
            pb = ps2.tile([RPC, CH], F32, tag="ps")
            nc.tensor.matmul(pb[:, 0:1], ones_r[:], wm[:],
                             start=True, stop=True)
            winv = const.tile([RPC, 1], F32)
            nc.vector.reciprocal(winv[:], pb[:, 0:1])
            sq_scale = const.tile([RPC, 1], F32)
            nc.vector.tensor_scalar_mul(sq_scale[:], winv[:], (Tf / BTf) ** 2)

            # ---- phase B: w matmul, s = sqrt(w*c), out = (r + lnb) * s ----
            # Wait-hinted so the static scheduler doesn't hoist these
            # matmuls into the sim window (the sim chain feeds lnb, which
            # gates every STT below).
            # Outputs alternate the two HW queues. A scalar-issued DMA waits
            # on its STT inside the ACT stream, but in steady state that STT
            # finished one cadence earlier, so no real stall.
            out_q = [nc.sync, nc.scalar]
            with tc.tile_wait_until(0.045):
                for k2, (a2, b2) in enumerate(chunks):
                    w2 = b2 - a2
                    ps_w = ps2.tile([RPC, CH], F32, tag="ps")
                    for o in range(0, w2, 512):
                        a = a2 + o
                        wp = a // WP
                        off = a - w_pieces[wp][0]
                        nc.tensor.matmul(ps_w[:, o:o + 512], labTc[:],
                                         wt[wp][:, off:off + 512],
                                         start=True, stop=True)
                    s = spool.tile([RPC, CH], BF16, tag="s")
                    nc.scalar.activation(s[:, 0:w2], ps_w[:, 0:w2], ACTF.Sqrt,
                                         scale=sq_scale[:])
                    rp = a2 // RP
                    ro = a2 - raw_pieces[rp][0]
                    st = stg.tile([RPC, CH], BF16, tag="st")
                    nc.vector.scalar_tensor_tensor(
                        st[:, 0:w2], raw[rp][:, ro:ro + w2],
                        lnb[:], s[:, 0:w2], op0=ALU.add, op1=ALU.mult)
                    out_q[k2 % 2].dma_start(out=out_d[:, a2:b2],
                                            in_=st[:, 0:w2])
    nc.compile()
    return nc


def _host_prep(query, keys, labels, queue, queue_label, Tf):
    bf16 = ml_dtypes.bfloat16
    f8 = ml_dtypes.float8_e4m3fn
    query = np.asarray(query, np.float32)
    keys = np.asarray(keys, np.float32)
    labels = np.asarray(labels, np.float32)
    queue = np.asarray(queue, np.float32)
    queue_label = np.asarray(queue_label, np.float32)

    # Pre-scaled by -1/T: the sim matmul then produces r = -raw/T directly,
    # letting the output stage fuse (lnb - raw/T)*s into one STT.
    qT = np.ascontiguousarray((query.T * (-1.0 / Tf)).astype(bf16))
    rhs_sim = np.concatenate([keys.T, queue], axis=1).astype(bf16)
    labT = np.ascontiguousarray(labels.T.astype(f8))          # [C, B] exact
    rhs_w = np.ascontiguousarray(
        np.concatenate([labels.T, queue_label], axis=1).astype(f8))

    in_maps = []
    idx = np.arange(RPC)
    for c in range(NCORES):
        rows = slice(c * RPC, (c + 1) * RPC)
        dmask = np.zeros((RPC, B), np.float32)
        dmask[idx, c * RPC + idx] = 1.0
        in_maps.append({
            "qT": np.ascontiguousarray(qT[:, rows]),
            "rhs_sim": rhs_sim,
            "labT": np.ascontiguousarray(labT[:, rows]),
            "rhs_w": rhs_w,
            "dmask": dmask.astype(bf16),
        })
    return in_maps


def _gather_output(results):
    return np.concatenate(
        [results[c]["out"] for c in range(NCORES)], axis=0).astype(np.float32)


def kernel(query, keys, labels, queue, queue_label, K, T, BT, **_unused):
    Tf = float(np.asarray(T))
    BTf = float(np.asarray(BT))
    nc = _build_nc(Tf, BTf)
    in_maps = _host_prep(query, keys, labels, queue, queue_label, Tf)
    res = run_bass_kernel_spmd(nc, in_maps, list(range(NCORES)))
    return _gather_output(res.results)


# Re-usable entry for test.py: returns (output, BassKernelResults) so the
# harness there can pull exec_time_ns / profile out of a traced run.
def kernel_traced(query, keys, labels, queue, queue_label, K, T, BT,
                  trace=False, **run_kwargs):
    Tf = float(np.asarray(T))
    BTf = float(np.asarray(BT))
    nc = _build_nc(Tf, BTf)
    in_maps = _host_prep(query, keys, labels, queue, queue_label, Tf)
    res = run_bass_kernel_spmd(nc, in_maps, list(range(NCORES)),
                               trace=trace, **run_kwargs)
    return _gather_output(res.results), res


# revision 37
# speedup vs baseline: 1.0541x; 1.0541x over previous
"""Trainium2 Bass kernel for nn_ContrastLoss (supervised-contrastive loss).

Reference computation (B=1024, D=128, C=100, K=32768, N=B+K=33792):
    l   = concat(labels, queue_label.T)          # [N, C]
    w   = labels @ l.T                           # [B, N] shared-class counts
    sim = query @ concat(keys, queue.T).T / T    # [B, N]
    logits = sim - rowmax(sim)
    denom  = sum(exp(logits) * logits_mask, 1)   # logits_mask zeros keys-diag
    loss = -(T/BT) * sqrt(w/max(w)) * (logits - log(denom))

Restructurings:
  * max(w) == max_i rowsum(labels_i) exactly (binary labels, diag of the
    keys block included): two tiny ones-vector matmuls + a free-dim max.
  * Constant softmax stabilizer m=1.0 (inputs are L2-normalized); the
    shift cancels in log_prob exactly.
  * qT host-scaled by -1/T so the sim matmul yields r = -raw/T and
    loss = s * (r + lnb), lnb = ln(denom') + m/T,
    s = sqrt(w*(T/BT)^2/wmax) >= 0 (folds sign and the w>0 mask).
    The elementwise tail is ONE all-bf16 scalar_tensor_tensor per chunk.
  * Output returned from device as bf16 (0.4% rel, gate is 2%): halves
    the dominant output DMA; host converts to f32.

Sharding: pure data-parallel over B -- core c owns rows [c*128,(c+1)*128)
and all N columns: NO collectives, immune to multi-core launch stagger.

Schedule notes (from NTFF traces): per-queue DMA sustains only ~115-160
GB/s, so inputs/outputs are spread over 4 queues (tensor/scalar queues
only carry dep-free transfers -- a dependent DMA's semaphore wait would
stall that engine's whole instruction stream).  2048-col chunks halve
the per-instruction semaphore overhead; one shared PSUM pool (2 x 4
banks) serves both phases.  A scheduler wait-hint keeps phase-B w
matmuls out of the sim-matmul window, because the sim chain feeds lnb
which gates every output STT.
"""

import numpy as np
import ml_dtypes

import concourse.bass as bass
import concourse.mybir as mybir
import concourse.tile as tile
from concourse import bacc
from concourse.bass_utils import run_bass_kernel_spmd

F32 = mybir.dt.float32
BF16 = mybir.dt.bfloat16
F8 = mybir.dt.float8e4
ALU = mybir.AluOpType
ACTF = mybir.ActivationFunctionType
AXX = mybir.AxisListType.X

B, D, C, KQ = 1024, 128, 100, 32768
NCORES = 8
RPC = B // NCORES          # 128 query rows per core
N = B + KQ                 # 33792 similarity columns, all on every core
CH = 2048                  # column chunk (psum tile: 4 banks)
SP = 4096                  # rhs_sim DMA piece (8KB bf16 partition lines)
RP = 4096                  # raw piece (cols)
WP = 8192                  # rhs_w DMA piece (8KB fp8 partition lines)
STAB = 1.0                 # constant softmax stabilizer


def _pieces(total, size):
    return [(a, min(a + size, total)) for a in range(0, total, size)]


def _build_nc(Tf: float, BTf: float):
    nc = bacc.Bacc("TRN2", target_bir_lowering=False, debug=False,
                   num_devices=NCORES)

    qT_d = nc.dram_tensor("qT", [D, RPC], BF16, kind="ExternalInput")
    rhs_sim_d = nc.dram_tensor("rhs_sim", [D, N], BF16, kind="ExternalInput")
    labT_d = nc.dram_tensor("labT", [C, RPC], F8, kind="ExternalInput")
    rhs_w_d = nc.dram_tensor("rhs_w", [C, N], F8, kind="ExternalInput")
    dmask_d = nc.dram_tensor("dmask", [RPC, B], BF16, kind="ExternalInput")
    out_d = nc.dram_tensor("out", [RPC, N], BF16, kind="ExternalOutput")

    sim_pieces = _pieces(N, SP)     # 9: 8x4096 + 1024
    raw_pieces = _pieces(N, RP)     # 9
    w_pieces = _pieces(N, WP)       # 5: 4x8192 + 1024
    chunks = _pieces(N, CH)         # 17: 16x2048 + 1024

    with tile.TileContext(nc) as tc:
        with (
            tc.tile_pool(name="const", bufs=1) as const,
            tc.tile_pool(name="simp", bufs=7) as simp,
            tc.tile_pool(name="spool", bufs=3) as spool,
            tc.tile_pool(name="stg", bufs=4) as stg,
            tc.tile_pool(name="ps2", bufs=2, space="PSUM") as ps2,
        ):
            # ---- input DMAs spread over 4 queues --------------------------
            # tensor/scalar queues get only dep-free transfers (their WAR
            # waits would stall the compute streams).
            # Only sync (SP) and scalar (Activation) are HW DGE queues; the
            # gpsimd queue is the slow software DGE (~60-100 GB/s), so it
            # gets only the late-needed rhs_w / dmask, while the latency-
            # critical rhs_sim pieces ride the two HW queues.
            qTc = const.tile([D, RPC], BF16)
            nc.sync.dma_start(out=qTc[:], in_=qT_d[:])
            labTc = const.tile([C, RPC], F8)
            nc.gpsimd.dma_start(out=labTc[:], in_=labT_d[:])

            wt = []
            for pi, (a, b) in enumerate(w_pieces):
                wt.append(const.tile([C, b - a], F8, name=f"wt{pi}"))
            nc.gpsimd.dma_start(out=wt[0][:], in_=rhs_w_d[:, 0:WP])

            st_tiles = []
            for pi, (a, b) in enumerate(sim_pieces):
                t = simp.tile([D, b - a], BF16, tag="sp", name=f"sp{pi}")
                eng = nc.scalar if pi % 2 == 0 else nc.sync
                eng.dma_start(out=t[:], in_=rhs_sim_d[:, a:b])
                st_tiles.append(t)

            dmask = const.tile([RPC, B], BF16)
            nc.gpsimd.dma_start(out=dmask[:], in_=dmask_d[:])
            for pi, (a, b) in enumerate(w_pieces):
                if pi == 0:
                    continue
                nc.gpsimd.dma_start(out=wt[pi][:], in_=rhs_w_d[:, a:b])

            ebias = const.tile([RPC, 1], F32)
            nc.vector.memset(ebias, -STAB / Tf)

            # ---- phase A: sim matmuls, DVE copy psum -> bf16 r = -raw/T ---
            raw = []
            for pi, (a, b) in enumerate(raw_pieces):
                raw.append(const.tile([D, b - a], BF16, name=f"raw{pi}"))
            for (a2, b2) in chunks:
                w2 = b2 - a2
                ps = ps2.tile([RPC, CH], F32, tag="ps")
                for o in range(0, w2, 512):
                    a = a2 + o
                    sp = next(i for i, (pa, pb) in enumerate(sim_pieces)
                              if pa <= a < pb)
                    off = a - sim_pieces[sp][0]
                    nc.tensor.matmul(ps[:, o:o + 512], qTc[:],
                                     st_tiles[sp][:, off:off + 512],
                                     start=True, stop=True)
                rp = a2 // RP
                ro = a2 - raw_pieces[rp][0]
                nc.vector.tensor_scalar_add(raw[rp][:, ro:ro + w2],
                                            ps[:, 0:w2], 0.0)

            # ---- all Exps (4096-col chunks over r), accum_out row sums ----
            e_keys = const.tile([RPC, B], F32)
            e_scr = const.tile([RPC, RP], BF16)
            acc = const.tile([RPC, len(raw_pieces) + 1], F32)
            nc.scalar.activation(e_keys[:], raw[0][:, 0:B], ACTF.Exp,
                                 bias=ebias[:], scale=-1.0,
                                 accum_out=acc[:, 0:1])
            nc.scalar.activation(e_scr[:, 0:RP - B], raw[0][:, B:RP], ACTF.Exp,
                                 bias=ebias[:], scale=-1.0,
                                 accum_out=acc[:, 1:2])
            for pi in range(1, len(raw_pieces)):
                a, b = raw_pieces[pi]
                nc.scalar.activation(e_scr[:, 0:b - a], raw[pi][:], ACTF.Exp,
                                     bias=ebias[:], scale=-1.0,
                                     accum_out=acc[:, pi + 1:pi + 2])

            # ---- denominator: subtract self-diagonal, take ln -------------
            nc.vector.tensor_mul(e_keys[:], e_keys[:], dmask[:])
            corr = const.tile([RPC, 1], F32)
            nc.vector.tensor_reduce(corr[:], e_keys[:], axis=AXX, op=ALU.add)
            dn = const.tile([RPC, 1], F32)
            nc.vector.tensor_reduce(dn[:], acc[:], axis=AXX, op=ALU.add)
            dn2 = const.tile([RPC, 1], F32)
            nc.vector.tensor_sub(dn2[:], dn[:], corr[:])
            lnd = const.tile([RPC, 1], F32)
            nc.scalar.activation(lnd[:], dn2[:], ACTF.Ln)
            lnb = const.tile([RPC, 1], F32)
            nc.vector.tensor_scalar_add(lnb[:], lnd[:], STAB / Tf)

            # ---- wmax = max_i rowsum(labels_i), via two tiny matmuls ------
            ones_c = const.tile([C, 1], F8)
            nc.vector.memset(ones_c, 1.0)
            ones_r = const.tile([1, RPC], F32)
            nc.vector.memset(ones_r, 1.0)
            pm = ps2.tile([RPC, CH], F32, tag="ps")
            nc.tensor.matmul(pm[0:1, 0:512], ones_c[:], wt[0][:, 0:512],
                             start=True, stop=True)
            nc.tensor.matmul(pm[0:1, 512:B], ones_c[:], wt[0][:, 512:B],
                             start=True, stop=True)
            wm = const.tile([1, 1], F32)
            nc.vector.tensor_reduce(wm[:], pm[0:1, 0:B], axis=AXX, op=ALU.max)
            pb = ps2.tile([RPC, CH], F32, tag="ps")
            nc.tensor.matmul(pb[:, 0:1], ones_r[:], wm[:],
                             start=True, stop=True)
            winv = const.tile([RPC, 1], F32)
            nc.vector.reciprocal(winv[:], pb[:, 0:1])
            sq_scale = const.tile([RPC, 1], F32)
            nc.vector.tensor_scalar_mul(sq_scale[:], winv[:], (Tf / BTf) ** 2)

            # ---- phase B: w matmul, s = sqrt(w*c), out = (r + lnb) * s ----
            # Wait-hinted so the static scheduler doesn't hoist these
            # matmuls into the sim window (the sim chain feeds lnb, which
            # gates every STT below).
            # Outputs alternate the two HW queues. A scalar-issued DMA waits
            # on its STT inside the ACT stream, but in steady state that STT
            # finished one cadence earlier, so no real stall.
            out_q = [nc.sync, nc.scalar]
            with tc.tile_wait_until(0.045):
                for k2, (a2, b2) in enumerate(chunks):
                    w2 = b2 - a2
                    ps_w = ps2.tile([RPC, CH], F32, tag="ps")
                    for o in range(0, w2, 512):
                        a = a2 + o
                        wp = a // WP
                        off = a - w_pieces[wp][0]
                        nc.tensor.matmul(ps_w[:, o:o + 512], labTc[:],
                                         wt[wp][:, off:off + 512],
                                         start=True, stop=True)
                    s = spool.tile([RPC, CH], BF16, tag="s")
                    nc.scalar.activation(s[:, 0:w2], ps_w[:, 0:w2], ACTF.Sqrt,
                                         scale=sq_scale[:])
                    rp = a2 // RP
                    ro = a2 - raw_pieces[rp][0]
                    st = stg.tile([RPC, CH], BF16, tag="st")
                    nc.vector.scalar_tensor_tensor(
                        st[:, 0:w2], raw[rp][:, ro:ro + w2],
                        lnb[:], s[:, 0:w2], op0=ALU.add, op1=ALU.mult)
                    out_q[k2 % 2].dma_start(out=out_d[:, a2:b2],
                                            in_=st[:, 0:w2])
    nc.compile()
    return nc


def _host_prep(query, keys, labels, queue, queue_label, Tf):
    bf16 = ml_dtypes.bfloat16
    f8 = ml_dtypes.float8_e4m3fn
    query = np.asarray(query, np.float32)
    keys = np.asarray(keys, np.float32)
    labels = np.asarray(labels, np.float32)
    queue = np.asarray(queue, np.float32)
    queue_label = np.asarray(queue_label, np.float32)

    # Pre-scaled by -1/T: the sim matmul then produces r = -raw/T directly,
    # letting the output stage fuse (lnb - raw/T)*s into one STT.
    qT = np.ascontiguousarray((query.T * (-1.0 / Tf)).astype(bf16))
    rhs_sim = np.concatenate([keys.T, queue], axis=1).astype(bf16)
    labT = np.ascontiguousarray(labels.T.astype(f8))          # [C, B] exact
    rhs_w = np.ascontiguousarray(
        np.concatenate([labels.T, queue_label], axis=1).astype(f8))

    in_maps = []
    idx = np.arange(RPC)
    for c in range(NCORES):
        rows = slice(c * RPC, (c + 1) * RPC)
        dmask = np.zeros((RPC, B), np.float32)
        dmask[idx, c * RPC + idx] = 1.0
        in_maps.append({
            "qT": np.ascontiguousarray(qT[:, rows]),
            "rhs_sim": rhs_sim,
            "labT": np.ascontiguousarray(labT[:, rows]),
            "rhs_w": rhs_w,
            "dmask": dmask.astype(bf16),
        })
    return in_maps


def _gather_output(results):
    return np.concatenate(
        [results[c]["out"] for c in range(NCORES)], axis=0).astype(np.float32)


def kernel(query, keys, labels, queue, queue_label, K, T, BT, **_unused):
    Tf = float(np.asarray(T))
    BTf = float(np.asarray(BT))
    nc = _build_nc(Tf, BTf)
    in_maps = _host_prep(query, keys, labels, queue, queue_label, Tf)
    res = run_bass_kernel_spmd(nc, in_maps, list(range(NCORES)))
    return _gather_output(res.results)


# Re-usable entry for test.py: returns (output, BassKernelResults) so the
# harness there can pull exec_time_ns / profile out of a traced run.
def kernel_traced(query, keys, labels, queue, queue_label, K, T, BT,
                  trace=False, **run_kwargs):
    Tf = float(np.asarray(T))
    BTf = float(np.asarray(BT))
    nc = _build_nc(Tf, BTf)
    in_maps = _host_prep(query, keys, labels, queue, queue_label, Tf)
    res = run_bass_kernel_spmd(nc, in_maps, list(range(NCORES)),
                               trace=trace, **run_kwargs)
    return _gather_output(res.results), res
